# revision 4
# baseline (speedup 1.0000x reference)
"""GNN message passing (gnn_message_passing) on 8 Trainium2 NeuronCores.

Computation (see reference):
    out = segment_sum over edges of  w[a] * vals[a,e] * x[src[a,e]]  into rows dst[a,e]
    out = gelu_exact(out / max(||out||_2, 1e-12))   (row-wise L2 normalize)

Strategy (node sharding, per the sharding hint):
  - Each of the 8 cores owns 6250 destination rows.
  - Host groups each core's incident edges by 128-row destination block,
    packs them into 128-edge tiles (edge p of tile g sits on partition p).
    Because dma_gather indices are int16, each block's edges are split into
    a "low" group (src < 32768) and "high" group (src >= 32768, gathered
    from a base-offset view of x).
  - Device, per block: two dma_gather calls pull the block's x[src] rows
    (fp16) from HBM into SBUF laid out [128 edges, T, 128 feat]; per tile a
    one-hot scatter matrix S[e, slot] = (iota==dloc[e]) * (w[a]*val[e]) is
    built by a single dual-op tensor_scalar on DVE; TensorE accumulates
    S^T @ X into a PSUM block of 128 output rows. Epilogue fuses
    L2-normalize + exact GELU on ScalarE/VectorE. No collectives needed -
    host concatenates the 8 per-core row shards.
"""

import sys

sys.path.insert(0, "/opt/trn_rl_repo")

import os
from contextlib import ExitStack

import numpy as np

import concourse.bass as bass
import concourse.tile as tile
from concourse import bacc, library_config, mybir
from concourse.bass_utils import run_bass_kernel_spmd

N_NODES = 50000
N_HID = 128
N_ADJ = 4
N_EDGE = 600000
N_CORES = 8
RPC = N_NODES // N_CORES          # 6250 destination rows per core
NBLK = (RPC + 127) // 128         # 49 blocks of 128 rows (last block 106 rows)
HALF = 32768                      # int16 index limit for dma_gather
EPS = 1e-12

fp16 = mybir.dt.float16
fp32 = mybir.dt.float32
i16 = mybir.dt.int16

LAST_RESULTS = None  # BassKernelResults of the most recent run (for test.py)


def _host_prep(x, weight, adj_src, adj_dst, adj_vals):
    """Partition + sort edges per (core, dst-block, src-half); build arrays."""
    x = np.ascontiguousarray(np.asarray(x, dtype=np.float32))
    weight = np.asarray(weight, dtype=np.float32).reshape(N_ADJ)
    src_f = np.asarray(adj_src, dtype=np.int64).reshape(-1)
    dst_f = np.asarray(adj_dst, dtype=np.int64).reshape(-1)
    val_f = np.asarray(adj_vals, dtype=np.float32).reshape(-1)
    aid_f = np.repeat(np.arange(N_ADJ, dtype=np.int64), N_EDGE)

    core = dst_f // RPC
    dloc = dst_f - core * RPC
    blk = dloc >> 7                 # dst block within core (0..NBLK-1)
    slot = dloc & 127               # dst slot within block (0..127)
    half = (src_f >= HALF).astype(np.int64)

    NG = NBLK * 2                   # (block, half) groups per core
    key = (core * NBLK + blk) * 2 + half
    order = np.lexsort((src_f, key))  # group by (core, blk, half), sort by src
    ks = key[order]

    counts = np.bincount(ks, minlength=N_CORES * NG)
    cnt = counts.reshape(N_CORES, NBLK, 2)
    # tiles per (block, half): shared across cores, padded to the max core
    Tg = np.maximum((cnt + 127) // 128, 1).max(axis=0).astype(np.int64)  # [NBLK,2]
    offs = np.zeros(NBLK * 2 + 1, dtype=np.int64)
    np.cumsum(Tg.reshape(-1), out=offs[1:])
    offs2 = offs.reshape  # noqa - kept simple below
    NT = int(offs[-1])

    # within-group rank of each (sorted) edge
    starts = np.zeros(N_CORES * NG, dtype=np.int64)
    np.cumsum(counts[:-1], out=starts[1:])
    r = np.arange(src_f.size, dtype=np.int64) - np.repeat(starts, counts)
    p = r & 127
    t = r >> 7
    core_s = ks // NG
    grp_s = ks % NG                 # (blk*2 + half)
    g = offs[grp_s] + t             # global tile column

    idx16 = np.zeros((N_CORES, 128, NT), dtype=np.int16)   # [p, tile] layout
    dmat = np.zeros((N_CORES, 128, NT), dtype=np.float32)
    v4 = np.zeros((N_CORES, 128, N_ADJ, NT), dtype=np.float16)

    src_rel = src_f[order] - (ks % 2) * HALF
    idx16[core_s, p, g] = src_rel.astype(np.int16)
    dmat[core_s, p, g] = slot[order].astype(np.float32)
    v4[core_s, p, aid_f[order], g] = val_f[order].astype(np.float16)

    # dma_gather idx layout per group: [16, T*8] wrap (idx j at [j%16, j//16]),
    # replicated to 128 partitions. Build the whole [128, NT*8] slab.
    idxw = np.zeros((N_CORES, 128, NT * 8), dtype=np.int16)
    for gi in range(NG):
        t0, t1 = int(offs[gi]), int(offs[gi + 1])
        Tn = t1 - t0
        n = Tn * 128
        # idx j (= t*128 + p) of this group -> [j%16, j//16]
        flat = idx16[:, :, t0:t1].transpose(0, 2, 1).reshape(N_CORES, n)  # j order
        wrapped = flat.reshape(N_CORES, n // 16, 16).transpose(0, 2, 1)  # [C,16,n/16]
        idxw[:, :, t0 * 8:t1 * 8] = np.tile(wrapped, (1, 8, 1))

    x16 = x.astype(np.float16)
    iota = np.tile(np.arange(128, dtype=np.float16), (128, 1))
    return x16, weight, idx16, idxw, dmat, v4, iota, Tg, offs, NT


def _build_program(Tg, offs, NT):
    """Build the single-core bass program (same for all 8 cores)."""
    nc = bacc.Bacc("TRN2", target_bir_lowering=False, debug=False)

    x_d = nc.dram_tensor("x16", [N_NODES, N_HID], fp16, kind="ExternalInput")
    w_d = nc.dram_tensor("w", [1, N_ADJ], fp32, kind="ExternalInput")
    idx_d = nc.dram_tensor("idxw", [128, NT * 8], i16, kind="ExternalInput")
    dmat_d = nc.dram_tensor("dmat", [128, NT], fp32, kind="ExternalInput")
    v4_d = nc.dram_tensor("v4", [128, N_ADJ * NT], fp16, kind="ExternalInput")
    iota_d = nc.dram_tensor("iota", [128, 128], fp16, kind="ExternalInput")
    out_d = nc.dram_tensor("out", [RPC, N_HID], fp32, kind="ExternalOutput")

    AF = mybir.ActivationFunctionType
    OP = mybir.AluOpType

    with tile.TileContext(nc) as tc, ExitStack() as ctx:
        meta = ctx.enter_context(tc.tile_pool(name="meta", bufs=1))
        gpool = ctx.enter_context(tc.tile_pool(name="gx", bufs=3))
        spool = ctx.enter_context(tc.tile_pool(name="s", bufs=6))
        ppool = ctx.enter_context(tc.tile_pool(name="psum", bufs=2, space="PSUM"))
        epool = ctx.enter_context(tc.tile_pool(name="epi", bufs=2))

        with tc.high_priority():
            nc.gpsimd.load_library(library_config.mlp)

        idx_sb = meta.tile([128, NT * 8], i16, tag="idx")
        nc.sync.dma_start(out=idx_sb[:], in_=idx_d[:])
        dmat_sb = meta.tile([128, NT], fp32, tag="dmat")
        nc.sync.dma_start(out=dmat_sb[:], in_=dmat_d[:])
        v4_sb = meta.tile([128, N_ADJ * NT], fp16, tag="v4")
        nc.sync.dma_start(out=v4_sb[:], in_=v4_d[:])
        iota_sb = meta.tile([128, 128], fp16, tag="iota")
        nc.sync.dma_start(out=iota_sb[:], in_=iota_d[:])

        # broadcast w[4] to all 128 partitions via a K=1 matmul with ones
        w1_sb = meta.tile([1, N_ADJ], fp32, tag="w1")
        nc.sync.dma_start(out=w1_sb[:], in_=w_d[:])
        ones_sb = meta.tile([1, 128], fp32, tag="ones")
        nc.vector.memset(ones_sb[:], 1.0)
        w_ps = ppool.tile([128, N_ADJ], fp32, space="PSUM", tag="wps")
        nc.tensor.matmul(out=w_ps[:], lhsT=ones_sb[:], rhs=w1_sb[:],
                         start=True, stop=True)
        w_bc = meta.tile([128, N_ADJ], fp32, tag="wbc")
        nc.vector.tensor_copy(w_bc[:], w_ps[:])

        # vs[p, g] = sum_a w[a] * v4[p, a, g]   (fp32)
        tmp0 = meta.tile([128, NT], fp32, tag="vs_tmp0")
        nc.vector.tensor_scalar(
            out=tmp0[:], in0=v4_sb[:, 0:NT], scalar1=w_bc[:, 0:1], scalar2=None,
            op0=OP.mult)
        tmp1 = meta.tile([128, NT], fp32, tag="vs_tmp1")
        nc.vector.scalar_tensor_tensor(
            out=tmp1[:], in0=v4_sb[:, NT:2 * NT], scalar=w_bc[:, 1:2],
            in1=tmp0[:], op0=OP.mult, op1=OP.add)
        nc.vector.scalar_tensor_tensor(
            out=tmp0[:], in0=v4_sb[:, 2 * NT:3 * NT], scalar=w_bc[:, 2:3],
            in1=tmp1[:], op0=OP.mult, op1=OP.add)
        vs_sb = meta.tile([128, NT], fp32, tag="vs")
        nc.vector.scalar_tensor_tensor(
            out=vs_sb[:], in0=v4_sb[:, 3 * NT:4 * NT], scalar=w_bc[:, 3:4],
            in1=tmp0[:], op0=OP.mult, op1=OP.add)

        for b in range(NBLK):
            Tlo = int(Tg[b, 0])
            Thi = int(Tg[b, 1])
            off_lo = int(offs[2 * b])
            off_hi = int(offs[2 * b + 1])
            gx_lo = gpool.tile([128, Tlo, N_HID], fp16, tag="gxlo")
            nc.gpsimd.dma_gather(
                out_ap=gx_lo[:], in_ap=x_d[:],
                idxs_ap=idx_sb[:, off_lo * 8:(off_lo + Tlo) * 8],
                num_idxs=Tlo * 128, num_idxs_reg=Tlo * 128, elem_size=N_HID,
                single_packet=False)
            gx_hi = gpool.tile([128, Thi, N_HID], fp16, tag="gxhi")
            nc.gpsimd.dma_gather(
                out_ap=gx_hi[:], in_ap=x_d[HALF:, :],
                idxs_ap=idx_sb[:, off_hi * 8:(off_hi + Thi) * 8],
                num_idxs=Thi * 128, num_idxs_reg=Thi * 128, elem_size=N_HID,
                single_packet=False)

            psum = ppool.tile([128, N_HID], fp32, space="PSUM", tag="acc")
            nt_b = Tlo + Thi
            for t in range(nt_b):
                if t < Tlo:
                    gcol = off_lo + t
                    rhs = gx_lo[:, t, :]
                else:
                    gcol = off_hi + (t - Tlo)
                    rhs = gx_hi[:, t - Tlo, :]
                S = spool.tile([128, 128], fp16, tag="S")
                nc.vector.tensor_scalar(
                    out=S[:], in0=iota_sb[:],
                    scalar1=dmat_sb[:, gcol:gcol + 1],
                    scalar2=vs_sb[:, gcol:gcol + 1],
                    op0=OP.is_equal, op1=OP.mult)
                nc.tensor.matmul(
                    out=psum[:], lhsT=S[:], rhs=rhs,
                    start=(t == 0), stop=(t == nt_b - 1))

            # epilogue: L2 normalize (eps=1e-12) + exact GELU
            sq = epool.tile([128, N_HID], fp32, tag="sq")
            ss = epool.tile([128, 1], fp32, tag="ss")
            nc.scalar.activation(out=sq[:], in_=psum[:], func=AF.Square,
                                 accum_out=ss[:])
            ssc = epool.tile([128, 1], fp32, tag="ssc")
            nc.vector.tensor_scalar(out=ssc[:], in0=ss[:], scalar1=float(EPS * EPS),
                                    scalar2=None, op0=OP.max)
            nrm = epool.tile([128, 1], fp32, tag="nrm")
            nc.scalar.sqrt(nrm[:], ssc[:])
            inv = epool.tile([128, 1], fp32, tag="inv")
            nc.vector.reciprocal(inv[:], nrm[:])
            res = epool.tile([128, N_HID], fp32, tag="res")
            nc.scalar.activation(out=res[:], in_=psum[:], func=AF.Gelu,
                                 scale=inv[:])
            rows = min(128, RPC - b * 128)
            nc.sync.dma_start(out=out_d[b * 128:b * 128 + rows, :],
                              in_=res[:rows, :])

    nc.compile()
    return nc


def kernel(x, weight, adj_src, adj_dst, adj_vals, _trace=None):
    global LAST_RESULTS
    x16, w, idx16, idxw, dmat, v4, iota, Tg, offs, NT = _host_prep(
        x, weight, adj_src, adj_dst, adj_vals)

    nc = _build_program(Tg, offs, NT)

    in_maps = []
    for c in range(N_CORES):
        in_maps.append({
            "x16": x16,
            "w": w.reshape(1, N_ADJ),
            "idxw": idxw[c],
            "dmat": dmat[c],
            "v4": v4[c].reshape(128, N_ADJ * NT),
            "iota": iota,
        })

    if _trace is None:
        _trace = bool(int(os.environ.get("GNN_TRACE", "0")))
    res = run_bass_kernel_spmd(nc, in_maps, list(range(N_CORES)), trace=_trace)
    LAST_RESULTS = res

    out = np.concatenate([res.results[c]["out"] for c in range(N_CORES)], axis=0)
    return out.astype(np.float32)


# revision 5
# speedup vs baseline: 3.0086x; 3.0086x over previous
"""GNN message passing (gnn_message_passing) on 8 Trainium2 NeuronCores.

Computation (see reference):
    out = segment_sum over edges of  w[a] * vals[a,e] * x[src[a,e]]  into rows dst[a,e]
    out = gelu_exact(out / max(||out||_2, 1e-12))   (row-wise L2 normalize)

Strategy (node sharding, per the sharding hint):
  - Each of the 8 cores owns 6250 destination rows.
  - Host groups each core's incident edges by 128-row destination block,
    packs them into 128-edge tiles (edge p of tile g sits on partition p).
    Because dma_gather indices are int16, each block's edges are split into
    a "low" group (src < 32768) and "high" group (src >= 32768, gathered
    from a base-offset view of x).
  - Device, per block: two dma_gather calls pull the block's x[src] rows
    (fp16) from HBM into SBUF laid out [128 edges, T, 128 feat]; per tile a
    one-hot scatter matrix S[e, slot] = (iota==dloc[e]) * (w[a]*val[e]) is
    built by a single dual-op tensor_scalar on DVE; TensorE accumulates
    S^T @ X into a PSUM block of 128 output rows. Epilogue fuses
    L2-normalize + exact GELU on ScalarE/VectorE. No collectives needed -
    host concatenates the 8 per-core row shards.
"""

import sys

sys.path.insert(0, "/opt/trn_rl_repo")

import os
from contextlib import ExitStack

import numpy as np

import concourse.bass as bass
import concourse.tile as tile
from concourse import bacc, library_config, mybir
from concourse.bass_utils import run_bass_kernel_spmd

N_NODES = 50000
N_HID = 128
N_ADJ = 4
N_EDGE = 600000
N_CORES = 8
RPC = N_NODES // N_CORES          # 6250 destination rows per core
NBLK = (RPC + 127) // 128         # 49 blocks of 128 rows (last block 106 rows)
HALF = 32768                      # int16 index limit for dma_gather
EPS = 1e-12

fp16 = mybir.dt.float16
fp32 = mybir.dt.float32
i16 = mybir.dt.int16

LAST_RESULTS = None  # BassKernelResults of the most recent run (for test.py)


def _host_prep(x, weight, adj_src, adj_dst, adj_vals):
    """Partition + sort edges per (core, dst-block, src-half); build arrays."""
    x = np.ascontiguousarray(np.asarray(x, dtype=np.float32))
    weight = np.asarray(weight, dtype=np.float32).reshape(N_ADJ)
    src_f = np.asarray(adj_src, dtype=np.int64).reshape(-1)
    dst_f = np.asarray(adj_dst, dtype=np.int64).reshape(-1)
    val_f = np.asarray(adj_vals, dtype=np.float32).reshape(-1)
    aid_f = np.repeat(np.arange(N_ADJ, dtype=np.int64), N_EDGE)

    core = dst_f // RPC
    dloc = dst_f - core * RPC
    blk = dloc >> 7                 # dst block within core (0..NBLK-1)
    slot = dloc & 127               # dst slot within block (0..127)
    half = (src_f >= HALF).astype(np.int64)

    NG = NBLK * 2                   # (block, half) groups per core
    key = (core * NBLK + blk) * 2 + half
    order = np.lexsort((src_f, key))  # group by (core, blk, half), sort by src
    ks = key[order]

    counts = np.bincount(ks, minlength=N_CORES * NG)
    cnt = counts.reshape(N_CORES, NBLK, 2)
    # tiles per (block, half): shared across cores, padded to the max core
    Tg = np.maximum((cnt + 127) // 128, 1).max(axis=0).astype(np.int64)  # [NBLK,2]
    offs = np.zeros(NBLK * 2 + 1, dtype=np.int64)
    np.cumsum(Tg.reshape(-1), out=offs[1:])
    offs2 = offs.reshape  # noqa - kept simple below
    NT = int(offs[-1])

    # within-group rank of each (sorted) edge
    starts = np.zeros(N_CORES * NG, dtype=np.int64)
    np.cumsum(counts[:-1], out=starts[1:])
    r = np.arange(src_f.size, dtype=np.int64) - np.repeat(starts, counts)
    p = r & 127
    t = r >> 7
    core_s = ks // NG
    grp_s = ks % NG                 # (blk*2 + half)
    g = offs[grp_s] + t             # global tile column

    idx16 = np.zeros((N_CORES, 128, NT), dtype=np.int16)   # [p, tile] layout
    dmat = np.zeros((N_CORES, 128, NT), dtype=np.float16)
    v4 = np.zeros((N_CORES, 128, N_ADJ, NT), dtype=np.float16)

    src_rel = src_f[order] - (ks % 2) * HALF
    idx16[core_s, p, g] = src_rel.astype(np.int16)
    dmat[core_s, p, g] = slot[order].astype(np.float16)
    v4[core_s, p, aid_f[order], g] = val_f[order].astype(np.float16)

    # dma_gather idx layout per group: [16, T*8] wrap (idx j at [j%16, j//16]),
    # replicated to 128 partitions. Build the whole [128, NT*8] slab.
    idxw = np.zeros((N_CORES, 128, NT * 8), dtype=np.int16)
    for gi in range(NG):
        t0, t1 = int(offs[gi]), int(offs[gi + 1])
        Tn = t1 - t0
        n = Tn * 128
        # idx j (= t*128 + p) of this group -> [j%16, j//16]
        flat = idx16[:, :, t0:t1].transpose(0, 2, 1).reshape(N_CORES, n)  # j order
        wrapped = flat.reshape(N_CORES, n // 16, 16).transpose(0, 2, 1)  # [C,16,n/16]
        idxw[:, :, t0 * 8:t1 * 8] = np.tile(wrapped, (1, 8, 1))

    x16 = x.astype(np.float16)
    iota = np.tile(np.arange(128, dtype=np.float16), (128, 1))
    return x16, weight, idx16, idxw, dmat, v4, iota, Tg, offs, NT


def _build_program(Tg, offs, NT):
    """Build the single-core bass program (same for all 8 cores)."""
    nc = bacc.Bacc("TRN2", target_bir_lowering=False, debug=False,
                   num_swdge_queues=4)

    x_d = nc.dram_tensor("x16", [N_NODES, N_HID], fp16, kind="ExternalInput")
    w_d = nc.dram_tensor("w", [1, N_ADJ], fp32, kind="ExternalInput")
    idx_d = nc.dram_tensor("idxw", [128, NT * 8], i16, kind="ExternalInput")
    dmat_d = nc.dram_tensor("dmat", [128, NT], fp16, kind="ExternalInput")
    v4_d = nc.dram_tensor("v4", [128, N_ADJ * NT], fp16, kind="ExternalInput")
    iota_d = nc.dram_tensor("iota", [128, 128], fp16, kind="ExternalInput")
    out_d = nc.dram_tensor("out", [RPC, N_HID], fp32, kind="ExternalOutput")

    AF = mybir.ActivationFunctionType
    OP = mybir.AluOpType

    with tile.TileContext(nc) as tc, ExitStack() as ctx:
        meta = ctx.enter_context(tc.tile_pool(name="meta", bufs=1))
        gpool = ctx.enter_context(tc.tile_pool(name="gx", bufs=3))
        spool = ctx.enter_context(tc.tile_pool(name="s", bufs=2))
        ppool = ctx.enter_context(tc.tile_pool(name="psum", bufs=2, space="PSUM"))
        epool = ctx.enter_context(tc.tile_pool(name="epi", bufs=2))

        with tc.high_priority():
            nc.gpsimd.load_library(library_config.mlp)

        idx_sb = meta.tile([128, NT * 8], i16, tag="idx")
        nc.sync.dma_start(out=idx_sb[:], in_=idx_d[:])
        dmat_sb = meta.tile([128, NT], fp16, tag="dmat")
        nc.sync.dma_start(out=dmat_sb[:], in_=dmat_d[:])
        v4_sb = meta.tile([128, N_ADJ * NT], fp16, tag="v4")
        nc.sync.dma_start(out=v4_sb[:], in_=v4_d[:])
        iota_sb = meta.tile([128, 128], fp16, tag="iota")
        nc.sync.dma_start(out=iota_sb[:], in_=iota_d[:])

        # broadcast w[4] to all 128 partitions via a K=1 matmul with ones
        w1_sb = meta.tile([1, N_ADJ], fp32, tag="w1")
        nc.sync.dma_start(out=w1_sb[:], in_=w_d[:])
        ones_sb = meta.tile([1, 128], fp32, tag="ones")
        nc.vector.memset(ones_sb[:], 1.0)
        w_ps = ppool.tile([128, N_ADJ], fp32, space="PSUM", tag="wps")
        nc.tensor.matmul(out=w_ps[:], lhsT=ones_sb[:], rhs=w1_sb[:],
                         start=True, stop=True)
        w_bc = meta.tile([128, N_ADJ], fp32, tag="wbc")
        nc.vector.tensor_copy(w_bc[:], w_ps[:])

        # vs[p, g] = sum_a w[a] * v4[p, a, g]   (fp32)
        tmp0 = meta.tile([128, NT], fp16, tag="vs_tmp0")
        nc.vector.tensor_scalar(
            out=tmp0[:], in0=v4_sb[:, 0:NT], scalar1=w_bc[:, 0:1], scalar2=None,
            op0=OP.mult)
        tmp1 = meta.tile([128, NT], fp16, tag="vs_tmp1")
        nc.vector.scalar_tensor_tensor(
            out=tmp1[:], in0=v4_sb[:, NT:2 * NT], scalar=w_bc[:, 1:2],
            in1=tmp0[:], op0=OP.mult, op1=OP.add)
        nc.vector.scalar_tensor_tensor(
            out=tmp0[:], in0=v4_sb[:, 2 * NT:3 * NT], scalar=w_bc[:, 2:3],
            in1=tmp1[:], op0=OP.mult, op1=OP.add)
        vs_sb = meta.tile([128, NT], fp16, tag="vs")
        nc.vector.scalar_tensor_tensor(
            out=vs_sb[:], in0=v4_sb[:, 3 * NT:4 * NT], scalar=w_bc[:, 3:4],
            in1=tmp0[:], op0=OP.mult, op1=OP.add)

        for b in range(NBLK):
            Tlo = int(Tg[b, 0])
            Thi = int(Tg[b, 1])
            off_lo = int(offs[2 * b])
            off_hi = int(offs[2 * b + 1])
            gx_lo = gpool.tile([128, Tlo, N_HID], fp16, tag="gxlo")
            nc.gpsimd.dma_gather(
                out_ap=gx_lo[:], in_ap=x_d[:],
                idxs_ap=idx_sb[:, off_lo * 8:(off_lo + Tlo) * 8],
                num_idxs=Tlo * 128, num_idxs_reg=Tlo * 128, elem_size=N_HID,
                single_packet=False, queue_num=(2 * b) % 4)
            gx_hi = gpool.tile([128, Thi, N_HID], fp16, tag="gxhi")
            nc.gpsimd.dma_gather(
                out_ap=gx_hi[:], in_ap=x_d[HALF:, :],
                idxs_ap=idx_sb[:, off_hi * 8:(off_hi + Thi) * 8],
                num_idxs=Thi * 128, num_idxs_reg=Thi * 128, elem_size=N_HID,
                single_packet=False, queue_num=(2 * b + 1) % 4)

            psum = ppool.tile([128, N_HID], fp32, space="PSUM", tag="acc")
            nt_b = Tlo + Thi
            # batched one-hot build: S[p, t, s] = (iota[s]==dloc[p,t]) * vs[p,t]
            # (off_hi == off_lo + Tlo, so columns [off_lo, off_lo+nt_b) cover both)
            S0 = spool.tile([128, nt_b, 128], fp16, tag="S0")
            nc.vector.tensor_tensor(
                out=S0[:],
                in0=iota_sb[:].rearrange("p (a f) -> p a f", a=1)
                    .to_broadcast([128, nt_b, 128]),
                in1=dmat_sb[:, off_lo:off_lo + nt_b].to_broadcast([128, nt_b, 128]),
                op=OP.is_equal)
            S = spool.tile([128, nt_b, 128], fp16, tag="S")
            nc.vector.tensor_tensor(
                out=S[:], in0=S0[:],
                in1=vs_sb[:, off_lo:off_lo + nt_b].to_broadcast([128, nt_b, 128]),
                op=OP.mult)
            for t in range(nt_b):
                if t < Tlo:
                    rhs = gx_lo[:, t, :]
                else:
                    rhs = gx_hi[:, t - Tlo, :]
                nc.tensor.matmul(
                    out=psum[:], lhsT=S[:, t, :], rhs=rhs,
                    start=(t == 0), stop=(t == nt_b - 1))

            # epilogue: L2 normalize (eps=1e-12) + exact GELU
            sq = epool.tile([128, N_HID], fp32, tag="sq")
            ss = epool.tile([128, 1], fp32, tag="ss")
            nc.scalar.activation(out=sq[:], in_=psum[:], func=AF.Square,
                                 accum_out=ss[:])
            ssc = epool.tile([128, 1], fp32, tag="ssc")
            nc.vector.tensor_scalar(out=ssc[:], in0=ss[:], scalar1=float(EPS * EPS),
                                    scalar2=None, op0=OP.max)
            nrm = epool.tile([128, 1], fp32, tag="nrm")
            nc.scalar.sqrt(nrm[:], ssc[:])
            inv = epool.tile([128, 1], fp32, tag="inv")
            nc.vector.reciprocal(inv[:], nrm[:])
            res = epool.tile([128, N_HID], fp32, tag="res")
            nc.scalar.activation(out=res[:], in_=psum[:], func=AF.Gelu,
                                 scale=inv[:])
            rows = min(128, RPC - b * 128)
            nc.sync.dma_start(out=out_d[b * 128:b * 128 + rows, :],
                              in_=res[:rows, :])

    nc.compile()
    return nc


def kernel(x, weight, adj_src, adj_dst, adj_vals, _trace=None):
    global LAST_RESULTS
    x16, w, idx16, idxw, dmat, v4, iota, Tg, offs, NT = _host_prep(
        x, weight, adj_src, adj_dst, adj_vals)

    nc = _build_program(Tg, offs, NT)

    in_maps = []
    for c in range(N_CORES):
        in_maps.append({
            "x16": x16,
            "w": w.reshape(1, N_ADJ),
            "idxw": idxw[c],
            "dmat": dmat[c],
            "v4": v4[c].reshape(128, N_ADJ * NT),
            "iota": iota,
        })

    if _trace is None:
        _trace = bool(int(os.environ.get("GNN_TRACE", "0")))
    res = run_bass_kernel_spmd(nc, in_maps, list(range(N_CORES)), trace=_trace)
    LAST_RESULTS = res

    out = np.concatenate([res.results[c]["out"] for c in range(N_CORES)], axis=0)
    return out.astype(np.float32)
